# revision 5
# baseline (speedup 1.0000x reference)
"""Trainium2 Bass kernel for nn_BLCD_Loss (retrieval_knn).

Math: for l2-normalized rows, ||a-b||^2 = 2 - 2*a.b, so all pairwise
distances come from two small Gram matmuls per core. The top-(K+1)
neighbor selection reduces to a per-row threshold (17th largest cosine)
found with two rounds of the DVE 8-wide `max` + `match_replace` ops, and
the neighbor gather becomes a 0/1 mask multiply. Self-pairs are excluded
up-front by subtracting a large constant on the (local) diagonal.

Sharding: 256 anchor rows -> 32 rows on each of 8 cores; each core gets
the full yi^T (256KB) plus its local slices, computes a partial scalar
loss; the host sums the 8 partials.
"""

import numpy as np

N, D, K = 256, 256, 16
M_MARGIN, T_THRESH, EPS = 0.6, 0.0025, 1e-12
NCORES, RPC = 8, 32  # cores, rows per core
BIG = 1000.0

_CACHE = {}


def _build():
    from concourse import bacc, mybir, tile
    import concourse.bass as bass

    dt = mybir.dt.float32
    Alu = mybir.AluOpType
    Act = mybir.ActivationFunctionType

    nc = bacc.Bacc("TRN2", target_bir_lowering=False, debug=False)

    yiT_d = nc.dram_tensor("yiT", [D, N], dt, kind="ExternalInput")
    yiLT_d = nc.dram_tensor("yiLT", [D, RPC], dt, kind="ExternalInput")
    yitT_d = nc.dram_tensor("yitT", [D, RPC], dt, kind="ExternalInput")
    yitl_d = nc.dram_tensor("yitl", [RPC, D], dt, kind="ExternalInput")
    eyeB_d = nc.dram_tensor("eyeB", [RPC, N], dt, kind="ExternalInput")
    out_d = nc.dram_tensor("out", [1, 1], dt, kind="ExternalOutput")

    with tile.TileContext(nc) as tc:
        with (
            tc.tile_pool(name="sb", bufs=1) as sb,
            tc.tile_pool(name="ps", bufs=1, space=bass.MemorySpace.PSUM) as ps,
        ):
            yiT0 = sb.tile([128, N], dt)
            yiT1 = sb.tile([128, N], dt)
            nc.sync.dma_start(yiT0[:], yiT_d[0:128, :])
            nc.sync.dma_start(yiT1[:], yiT_d[128:256, :])
            yiLT0 = sb.tile([128, RPC], dt)
            yiLT1 = sb.tile([128, RPC], dt)
            nc.sync.dma_start(yiLT0[:], yiLT_d[0:128, :])
            nc.sync.dma_start(yiLT1[:], yiLT_d[128:256, :])
            yitT0 = sb.tile([128, RPC], dt)
            yitT1 = sb.tile([128, RPC], dt)
            nc.sync.dma_start(yitT0[:], yitT_d[0:128, :])
            nc.sync.dma_start(yitT1[:], yitT_d[128:256, :])
            yitl = sb.tile([RPC, D], dt)
            nc.sync.dma_start(yitl[:], yitl_d[:, :])
            eyeB = sb.tile([RPC, N], dt)
            nc.sync.dma_start(eyeB[:], eyeB_d[:, :])
            ones = sb.tile([128, RPC], dt)
            nc.vector.memset(ones[:], 1.0)
            cEPS = sb.tile([128, 1], dt)
            nc.vector.memset(cEPS[:], EPS)
            cHALF = sb.tile([128, 1], dt)
            nc.vector.memset(cHALF[:], 0.5)

            # ---- column norms of yi: s_j = sum_d yi[j,d]^2 via ones-matmul
            sq0 = sb.tile([128, N], dt)
            sq1 = sb.tile([128, N], dt)
            nc.scalar.square(sq0[:], yiT0[:])
            nc.scalar.square(sq1[:], yiT1[:])
            ps_s = ps.tile([1, N], dt)
            nc.tensor.matmul(ps_s[:], ones[:, 0:1], sq0[:], start=True, stop=False)
            nc.tensor.matmul(ps_s[:], ones[:, 0:1], sq1[:], start=False, stop=True)
            t_row = sb.tile([1, N], dt)
            nc.scalar.activation(t_row[:], ps_s[:], Act.Sqrt, bias=cEPS[0:1, :], scale=1.0)
            inv_row = sb.tile([1, N], dt)
            nc.vector.reciprocal(inv_row[:], t_row[:])
            # broadcast inv_row down 32 partitions via K=1 matmul
            ps_b = ps.tile([RPC, N], dt)
            nc.tensor.matmul(ps_b[:], ones[0:1, 0:RPC], inv_row[:], start=True, stop=True)

            # ---- raw Gram matrices (local rows x all)
            ps_R = ps.tile([RPC, N], dt)
            nc.tensor.matmul(ps_R[:], yiLT0[:], yiT0[:], start=True, stop=False)
            nc.tensor.matmul(ps_R[:], yiLT1[:], yiT1[:], start=False, stop=True)
            ps_Rt = ps.tile([RPC, N], dt)
            nc.tensor.matmul(ps_Rt[:], yitT0[:], yiT0[:], start=True, stop=False)
            nc.tensor.matmul(ps_Rt[:], yitT1[:], yiT1[:], start=False, stop=True)

            # ---- local row norms: diag(R) via eyeB mask (value BIG on diag)
            scrA = sb.tile([RPC, N], dt)
            nc.vector.tensor_tensor(scrA[:], ps_R[:], eyeB[:], op=Alu.mult)
            sL = sb.tile([RPC, 1], dt)
            nc.vector.tensor_reduce(sL[:], scrA[:], axis=mybir.AxisListType.X, op=Alu.add)
            t_loc = sb.tile([RPC, 1], dt)
            nc.scalar.activation(t_loc[:], sL[:], Act.Sqrt, bias=cEPS[0:RPC, :], scale=1.0 / BIG)
            inv_loc = sb.tile([RPC, 1], dt)
            nc.vector.reciprocal(inv_loc[:], t_loc[:])
            sc_loc = sb.tile([RPC, 1], dt)
            nc.vector.tensor_scalar_mul(sc_loc[:], inv_loc[:], -0.5)

            # ---- norms of local yi_t rows
            scrB = sb.tile([RPC, D], dt)
            nc.vector.tensor_tensor(scrB[:], yitl[:], yitl[:], op=Alu.mult)
            sT = sb.tile([RPC, 1], dt)
            nc.vector.tensor_reduce(sT[:], scrB[:], axis=mybir.AxisListType.X, op=Alu.add)
            t_t = sb.tile([RPC, 1], dt)
            nc.scalar.activation(t_t[:], sT[:], Act.Sqrt, bias=cEPS[0:RPC, :], scale=1.0)
            inv_t = sb.tile([RPC, 1], dt)
            nc.vector.reciprocal(inv_t[:], t_t[:])
            sc_t = sb.tile([RPC, 1], dt)
            nc.vector.tensor_scalar_mul(sc_t[:], inv_t[:], -0.5)
            sc_tB = sb.tile([RPC, 1], dt)
            nc.vector.tensor_scalar_mul(sc_tB[:], inv_t[:], -0.5 / BIG)

            # ---- column-normalized Grams (row scale folded into ACT later)
            # (compiler rejects two PSUM operands in one TensorTensor)
            b_sb = sb.tile([RPC, N], dt)
            nc.vector.tensor_copy(b_sb[:], ps_b[:])
            G1 = sb.tile([RPC, N], dt)
            nc.vector.tensor_tensor(G1[:], ps_R[:], b_sb[:], op=Alu.mult)
            H1 = sb.tile([RPC, N], dt)
            nc.vector.tensor_tensor(H1[:], ps_Rt[:], b_sb[:], op=Alu.mult)
            work = sb.tile([RPC, N], dt)
            nc.vector.tensor_sub(work[:], G1[:], eyeB[:])

            # dis[i,j] = 0.5*sqrt(2-2*cos) = sqrt(-0.5*inv_i*G1 + 0.5)
            dis = sb.tile([RPC, N], dt)
            nc.scalar.activation(dis[:], work[:], Act.Sqrt, bias=cHALF[0:RPC, :], scale=sc_loc[:])
            dis_t = sb.tile([RPC, N], dt)
            nc.scalar.activation(dis_t[:], H1[:], Act.Sqrt, bias=cHALF[0:RPC, :], scale=sc_t[:])

            # ---- top-16 neighbor threshold per row (self already pushed low)
            m1 = sb.tile([RPC, 8], dt)
            nc.vector.max(out=m1[:], in_=work[:])
            w2 = sb.tile([RPC, N], dt)
            nc.vector.match_replace(
                out=w2[:], in_to_replace=m1[:], in_values=work[:], imm_value=-BIG
            )
            m2 = sb.tile([RPC, 8], dt)
            nc.vector.max(out=m2[:], in_=w2[:])
            mask = sb.tile([RPC, N], dt)
            nc.vector.tensor_scalar(
                mask[:], work[:], m2[:, 7:8], None, op0=Alu.is_ge
            )

            # ---- e1 = sum over neighbors of (dis - dis_t)^2
            diff = sb.tile([RPC, N], dt)
            nc.vector.tensor_sub(diff[:], dis[:], dis_t[:])
            mdiff = sb.tile([RPC, N], dt)
            nc.vector.tensor_tensor(mdiff[:], diff[:], mask[:], op=Alu.mult)
            scrC = sb.tile([RPC, N], dt)
            nc.vector.tensor_tensor(scrC[:], mdiff[:], mdiff[:], op=Alu.mult)
            e1row = sb.tile([RPC, 1], dt)
            nc.vector.tensor_reduce(e1row[:], scrC[:], axis=mybir.AxisListType.X, op=Alu.add)

            # ---- e2 = sum relu(dis(yi,yit) + margin - second_nn)
            scrD = sb.tile([RPC, N], dt)
            nc.vector.tensor_tensor(scrD[:], H1[:], eyeB[:], op=Alu.mult)
            hd2 = sb.tile([RPC, 1], dt)
            nc.vector.tensor_reduce(hd2[:], scrD[:], axis=mybir.AxisListType.X, op=Alu.add)
            dis_ii = sb.tile([RPC, 1], dt)
            nc.scalar.activation(dis_ii[:], hd2[:], Act.Sqrt, bias=cHALF[0:RPC, :], scale=sc_tB[:])
            dis2 = sb.tile([RPC, 1], dt)
            nc.scalar.activation(dis2[:], m1[:, 0:1], Act.Sqrt, bias=cHALF[0:RPC, :], scale=sc_loc[:])
            bias2 = sb.tile([RPC, 1], dt)
            nc.vector.tensor_scalar(
                bias2[:], dis2[:], -1.0, M_MARGIN, op0=Alu.mult, op1=Alu.add
            )
            e2row = sb.tile([RPC, 1], dt)
            nc.scalar.activation(e2row[:], dis_ii[:], Act.Relu, bias=bias2[:], scale=1.0)

            # ---- combine + partition-reduce via ones-matmul
            tot = sb.tile([RPC, 1], dt)
            nc.vector.tensor_add(tot[:], e1row[:], e2row[:])
            ps_f = ps.tile([1, 1], dt)
            nc.tensor.matmul(ps_f[:], ones[0:RPC, 0:1], tot[:], start=True, stop=True)
            outsb = sb.tile([1, 1], dt)
            nc.vector.tensor_scalar_add(outsb[:], ps_f[:], -float(RPC * K * T_THRESH))
            nc.sync.dma_start(out_d[:], outsb[:])

    nc.compile()
    return nc


def _in_maps(yi, yi_t):
    yi = np.ascontiguousarray(np.asarray(yi, np.float32))
    yi_t = np.ascontiguousarray(np.asarray(yi_t, np.float32))
    yiT = np.ascontiguousarray(yi.T)
    maps = []
    for c in range(NCORES):
        r0 = c * RPC
        eyeB = np.zeros((RPC, N), np.float32)
        eyeB[np.arange(RPC), r0 + np.arange(RPC)] = BIG
        maps.append({
            "yiT": yiT,
            "yiLT": np.ascontiguousarray(yi[r0:r0 + RPC].T),
            "yitT": np.ascontiguousarray(yi_t[r0:r0 + RPC].T),
            "yitl": np.ascontiguousarray(yi_t[r0:r0 + RPC]),
            "eyeB": eyeB,
        })
    return maps


def kernel(yi, yi_t):
    from concourse.bass_utils import run_bass_kernel_spmd

    if "nc" not in _CACHE:
        _CACHE["nc"] = _build()
    nc = _CACHE["nc"]
    res = run_bass_kernel_spmd(nc, _in_maps(yi, yi_t), list(range(NCORES)))
    partials = [res.results[c]["out"][0, 0] for c in range(NCORES)]
    return np.float32(np.sum(partials, dtype=np.float64))


# revision 8
# speedup vs baseline: 1.0053x; 1.0053x over previous
"""Trainium2 Bass kernel for nn_BLCD_Loss (retrieval_knn).

Math: for l2-normalized rows, ||a-b||^2 = 2 - 2*a.b, so all pairwise
distances come from two small Gram matmuls per core. The top-(K+1)
neighbor selection reduces to a per-row threshold (17th largest cosine)
found with two rounds of the DVE 8-wide `max` + `match_replace` ops, and
the neighbor gather becomes a 0/1 mask multiply. Self-pairs are excluded
up-front by subtracting a large constant on the (local) diagonal.

Sharding: 256 anchor rows -> 32 rows on each of 8 cores; each core gets
the full yi^T (256KB) plus its local slices, computes a partial scalar
loss; the host sums the 8 partials.
"""

import numpy as np

N, D, K = 256, 256, 16
M_MARGIN, T_THRESH, EPS = 0.6, 0.0025, 1e-12
NCORES, RPC = 8, 32  # cores, rows per core
BIG = 1000.0

_CACHE = {}


def _build():
    from concourse import bacc, mybir, tile
    import concourse.bass as bass

    dt = mybir.dt.float32
    Alu = mybir.AluOpType
    Act = mybir.ActivationFunctionType

    nc = bacc.Bacc("TRN2", target_bir_lowering=False, debug=False)

    yiT_d = nc.dram_tensor("yiT", [D, N], dt, kind="ExternalInput")
    yiLT_d = nc.dram_tensor("yiLT", [D, RPC], dt, kind="ExternalInput")
    yitT_d = nc.dram_tensor("yitT", [D, RPC], dt, kind="ExternalInput")
    ylcat_d = nc.dram_tensor("ylcat", [RPC, 2 * D], dt, kind="ExternalInput")
    eyeB_d = nc.dram_tensor("eyeB", [RPC, N], dt, kind="ExternalInput")
    i32_d = nc.dram_tensor("i32", [RPC, RPC], dt, kind="ExternalInput")
    eyeN_d = nc.dram_tensor("eyeN", [RPC, N], dt, kind="ExternalInput")
    out_d = nc.dram_tensor("out", [1, 1], dt, kind="ExternalOutput")

    with tile.TileContext(nc) as tc:
        with (
            tc.tile_pool(name="sb", bufs=1) as sb,
            tc.tile_pool(name="ps", bufs=1, space=bass.MemorySpace.PSUM) as ps,
        ):
            yiT0 = sb.tile([128, N], dt)
            yiT1 = sb.tile([128, N], dt)
            nc.sync.dma_start(yiT0[:], yiT_d[0:128, :])
            nc.sync.dma_start(yiT1[:], yiT_d[128:256, :])
            yiLT0 = sb.tile([128, RPC], dt)
            yiLT1 = sb.tile([128, RPC], dt)
            nc.sync.dma_start(yiLT0[:], yiLT_d[0:128, :])
            nc.sync.dma_start(yiLT1[:], yiLT_d[128:256, :])
            yitT0 = sb.tile([128, RPC], dt)
            yitT1 = sb.tile([128, RPC], dt)
            nc.sync.dma_start(yitT0[:], yitT_d[0:128, :])
            nc.sync.dma_start(yitT1[:], yitT_d[128:256, :])
            ylcat = sb.tile([RPC, 2 * D], dt)
            nc.sync.dma_start(ylcat[:], ylcat_d[:, :])
            eyeB = sb.tile([RPC, N], dt)
            nc.sync.dma_start(eyeB[:], eyeB_d[:, :])
            i32 = sb.tile([RPC, RPC], dt)
            nc.sync.dma_start(i32[:], i32_d[:, :])
            eyeN = sb.tile([RPC, N], dt)
            nc.sync.dma_start(eyeN[:], eyeN_d[:, :])
            ones = sb.tile([128, RPC], dt)
            nc.vector.memset(ones[:], 1.0)
            cEPS = sb.tile([128, 1], dt)
            nc.vector.memset(cEPS[:], EPS)
            cHALF = sb.tile([128, 1], dt)
            nc.vector.memset(cHALF[:], 0.5)

            # ---- column norms of yi: s_j = sum_d yi[j,d]^2 via ones-matmul
            sq0 = sb.tile([128, N], dt)
            sq1 = sb.tile([128, N], dt)
            nc.scalar.square(sq0[:], yiT0[:])
            nc.scalar.square(sq1[:], yiT1[:])
            ps_s = ps.tile([1, N], dt)
            nc.tensor.matmul(ps_s[:], ones[:, 0:1], sq0[:], start=True, stop=False)
            nc.tensor.matmul(ps_s[:], ones[:, 0:1], sq1[:], start=False, stop=True)
            t_row = sb.tile([1, N], dt)
            nc.scalar.activation(t_row[:], ps_s[:], Act.Sqrt, bias=cEPS[0:1, :], scale=1.0)
            inv_row = sb.tile([1, N], dt)
            nc.vector.reciprocal(inv_row[:], t_row[:])
            # broadcast inv_row down 32 partitions via K=1 matmul
            ps_b = ps.tile([RPC, N], dt)
            nc.tensor.matmul(ps_b[:], ones[0:1, 0:RPC], inv_row[:], start=True, stop=True)

            # ---- raw Gram matrices (local rows x all)
            ps_R = ps.tile([RPC, N], dt)
            nc.tensor.matmul(ps_R[:], yiLT0[:], yiT0[:], start=True, stop=False)
            nc.tensor.matmul(ps_R[:], yiLT1[:], yiT1[:], start=False, stop=False)
            nc.tensor.matmul(ps_R[:], i32[:], eyeN[:], start=False, stop=True)
            ps_Rt = ps.tile([RPC, N], dt)
            nc.tensor.matmul(ps_Rt[:], yitT0[:], yiT0[:], start=True, stop=False)
            nc.tensor.matmul(ps_Rt[:], yitT1[:], yiT1[:], start=False, stop=True)

            # ---- norms of local yi and yi_t rows in one TT+reduce pass
            scrN = sb.tile([RPC, 2 * D], dt)
            nc.vector.tensor_tensor(scrN[:], ylcat[:], ylcat[:], op=Alu.mult)
            nrm2 = sb.tile([RPC, 2], dt)
            nc.vector.tensor_reduce(
                nrm2[:], scrN[:].rearrange("p (g x) -> p g x", g=2),
                axis=mybir.AxisListType.X, op=Alu.add)
            t2 = sb.tile([RPC, 2], dt)
            nc.scalar.activation(t2[:], nrm2[:], Act.Sqrt, bias=cEPS[0:RPC, :], scale=1.0)
            inv2 = sb.tile([RPC, 2], dt)
            nc.vector.reciprocal(inv2[:], t2[:])
            sc_loc = sb.tile([RPC, 1], dt)
            nc.vector.tensor_scalar_mul(sc_loc[:], inv2[:, 0:1], -0.5)
            sc_t = sb.tile([RPC, 1], dt)
            nc.vector.tensor_scalar_mul(sc_t[:], inv2[:, 1:2], -0.5)
            sc_tB = sb.tile([RPC, 1], dt)
            nc.vector.tensor_scalar_mul(sc_tB[:], inv2[:, 1:2], -0.5 / BIG)

            # ---- column-normalized Grams (row scale folded into ACT later)
            # (compiler rejects two PSUM operands in one TensorTensor)
            b_sb = sb.tile([RPC, N], dt)
            nc.vector.tensor_copy(b_sb[:], ps_b[:])
            work = sb.tile([RPC, N], dt)
            nc.vector.tensor_tensor(work[:], ps_R[:], b_sb[:], op=Alu.mult)
            H1 = sb.tile([RPC, N], dt)
            nc.vector.tensor_tensor(H1[:], ps_Rt[:], b_sb[:], op=Alu.mult)

            # dis[i,j] = 0.5*sqrt(2-2*cos) = sqrt(-0.5*inv_i*G1 + 0.5)
            dis = sb.tile([RPC, N], dt)
            nc.scalar.activation(dis[:], work[:], Act.Sqrt, bias=cHALF[0:RPC, :], scale=sc_loc[:])
            dis_t = sb.tile([RPC, N], dt)
            nc.scalar.activation(dis_t[:], H1[:], Act.Sqrt, bias=cHALF[0:RPC, :], scale=sc_t[:])

            # ---- top-16 neighbor threshold per row (self already pushed low)
            m1 = sb.tile([RPC, 8], dt)
            nc.vector.max(out=m1[:], in_=work[:])
            w2 = sb.tile([RPC, N], dt)
            nc.vector.match_replace(
                out=w2[:], in_to_replace=m1[:], in_values=work[:], imm_value=-BIG
            )
            m2 = sb.tile([RPC, 8], dt)
            nc.vector.max(out=m2[:], in_=w2[:])
            mask = sb.tile([RPC, N], dt)
            nc.vector.tensor_scalar(
                mask[:], work[:], m2[:, 7:8], None, op0=Alu.is_ge
            )

            # ---- e1 = sum over neighbors of (dis - dis_t)^2
            diff = sb.tile([RPC, N], dt)
            nc.vector.tensor_sub(diff[:], dis[:], dis_t[:])
            mdiff = sb.tile([RPC, N], dt)
            nc.vector.tensor_tensor(mdiff[:], diff[:], mask[:], op=Alu.mult)
            scrC = sb.tile([RPC, N], dt)
            nc.vector.tensor_tensor(scrC[:], mdiff[:], mdiff[:], op=Alu.mult)
            e1row = sb.tile([RPC, 1], dt)
            nc.vector.tensor_reduce(e1row[:], scrC[:], axis=mybir.AxisListType.X, op=Alu.add)

            # ---- e2 = sum relu(dis(yi,yit) + margin - second_nn)
            scrD = sb.tile([RPC, N], dt)
            nc.vector.tensor_tensor(scrD[:], H1[:], eyeB[:], op=Alu.mult)
            hd2 = sb.tile([RPC, 1], dt)
            nc.vector.tensor_reduce(hd2[:], scrD[:], axis=mybir.AxisListType.X, op=Alu.add)
            dis_ii = sb.tile([RPC, 1], dt)
            nc.scalar.activation(dis_ii[:], hd2[:], Act.Sqrt, bias=cHALF[0:RPC, :], scale=sc_tB[:])
            dis2 = sb.tile([RPC, 1], dt)
            nc.scalar.activation(dis2[:], m1[:, 0:1], Act.Sqrt, bias=cHALF[0:RPC, :], scale=sc_loc[:])
            bias2 = sb.tile([RPC, 1], dt)
            nc.vector.tensor_scalar(
                bias2[:], dis2[:], -1.0, M_MARGIN, op0=Alu.mult, op1=Alu.add
            )
            e2row = sb.tile([RPC, 1], dt)
            nc.scalar.activation(e2row[:], dis_ii[:], Act.Relu, bias=bias2[:], scale=1.0)

            # ---- combine + partition-reduce via ones-matmul
            tot = sb.tile([RPC, 1], dt)
            nc.vector.tensor_add(tot[:], e1row[:], e2row[:])
            ps_f = ps.tile([1, 1], dt)
            nc.tensor.matmul(ps_f[:], ones[0:RPC, 0:1], tot[:], start=True, stop=True)
            outsb = sb.tile([1, 1], dt)
            nc.vector.tensor_scalar_add(outsb[:], ps_f[:], -float(RPC * K * T_THRESH))
            nc.sync.dma_start(out_d[:], outsb[:])

    nc.compile()
    return nc


def _in_maps(yi, yi_t):
    yi = np.ascontiguousarray(np.asarray(yi, np.float32))
    yi_t = np.ascontiguousarray(np.asarray(yi_t, np.float32))
    yiT = np.ascontiguousarray(yi.T)
    maps = []
    for c in range(NCORES):
        r0 = c * RPC
        eyeB = np.zeros((RPC, N), np.float32)
        eyeB[np.arange(RPC), r0 + np.arange(RPC)] = BIG
        maps.append({
            "yiT": yiT,
            "yiLT": np.ascontiguousarray(yi[r0:r0 + RPC].T),
            "yitT": np.ascontiguousarray(yi_t[r0:r0 + RPC].T),
            "ylcat": np.ascontiguousarray(
                np.hstack([yi[r0:r0 + RPC], yi_t[r0:r0 + RPC]])),
            "eyeB": eyeB,
            "i32": np.eye(RPC, dtype=np.float32),
            "eyeN": -eyeB,
        })
    return maps


def kernel(yi, yi_t):
    from concourse.bass_utils import run_bass_kernel_spmd

    if "nc" not in _CACHE:
        _CACHE["nc"] = _build()
    nc = _CACHE["nc"]
    res = run_bass_kernel_spmd(nc, _in_maps(yi, yi_t), list(range(NCORES)))
    partials = [res.results[c]["out"][0, 0] for c in range(NCORES)]
    return np.float32(np.sum(partials, dtype=np.float64))
